# revision 58
# baseline (speedup 1.0000x reference)
"""Trainium2 Bass kernel for the LocalGNOBlock (windowed GNN message passing).

Math restructuring (vs the naive 12x full MLP evaluations):
  msg first layer is linear over concat([h_i, h_j, dc]):
      z_d[i] = (A - C)[i] + (B + C)[i+d] + b1,  d in {+-1..+-6}
  where A = h @ W1a, B = h @ W1b, C = coord x w1c (rank-1).
  Interior chunks fold the whole message-2nd-layer + U1b product:
      u += sum_d silu(z_d) @ (W2/12 @ U1b)     (12 matmuls, PSUM accum)
  so the "agg" tensor is never materialized except at the two boundary
  chunks (count fixup).  LayerNorm stats are per-token (channel dim on
  partitions) via band-select ones matmuls packed into one PSUM bank in
  two half-batches, so normalization of the first half overlaps pass-1
  compute of the second half.

Engine budget per 512-token chunk (targets):
  ACT   silu(12T) 5.4us + silu(s2) 0.7us            -> floor ~6.1us
  DVE   z-build 3.4 + E/D casts 1.4 + x 0.7 + norm  -> ~5.5-6.9us
  PE    ~22 matmuls x 215ns (warm clock)            -> ~5us
  GPSIMD x^2 (SBUF only - no PSUM port)             -> ~1.9us
  D_B shifted copy runs as SBUF->SBUF DMA.

Sharding: batch dim B=8 -> one batch element per NeuronCore (no halo).
Host pre/post: transpose h -> [128, N] per core, transpose back after.
"""

import numpy as np

K = 6
HID = 128
N = 16384
B = 8
EPS = 1e-5
T = 512                 # token chunk (matmul + elementwise granularity)
NCH = N // T            # 32 chunks
NHALF = NCH // 2        # stats half-batch
OFF0 = 8                # D_full column of token 0 (even, for fp16 alignment)
NCOL = N + 2 * OFF0     # D_full width

# z segment order: A-block = D_A even offsets (segs 0-5), B-block = D_B odd
# offsets (segs 6-11); each block is one DVE build op + one silu op, so the
# A-block message matmuls release as soon as silu-A completes.
SEG_ORDER = [-6, -4, -2, 2, 4, 6, -5, -3, -1, 1, 3, 5]

_compiled = None


def _build_bass(dt_act):
    import concourse.bacc as bacc
    import concourse.bass as bass
    import concourse.tile as tile
    from concourse import mybir

    f32 = mybir.dt.float32
    DT = dt_act

    nc = bacc.Bacc("TRN2", target_bir_lowering=False, debug=False)

    # ---- DRAM I/O ----
    hT = nc.dram_tensor("hT", [HID, N], DT, kind="ExternalInput")
    coordR = nc.dram_tensor("coordR", [1, N], DT, kind="ExternalInput")
    W1a = nc.dram_tensor("W1a", [HID, HID], DT, kind="ExternalInput")
    W1b = nc.dram_tensor("W1b", [HID, HID], DT, kind="ExternalInput")
    w1c = nc.dram_tensor("w1c", [1, HID], DT, kind="ExternalInput")      # +w1c
    w1cn = nc.dram_tensor("w1cn", [1, HID], DT, kind="ExternalInput")    # -w1c
    W2s = nc.dram_tensor("W2s", [HID, HID], DT, kind="ExternalInput")    # W2/12
    W2u = nc.dram_tensor("W2u", [HID, HID], DT, kind="ExternalInput")    # W2/12@U1b
    U1a = nc.dram_tensor("U1a", [HID, HID], DT, kind="ExternalInput")
    U1b = nc.dram_tensor("U1b", [HID, HID], DT, kind="ExternalInput")
    U2 = nc.dram_tensor("U2", [HID, HID], DT, kind="ExternalInput")
    ident = nc.dram_tensor("ident", [HID, HID], DT, kind="ExternalInput")
    b1c = nc.dram_tensor("b1c", [HID, 1], f32, kind="ExternalInput")      # msg_b1
    buc = nc.dram_tensor("buc", [HID, 1], f32, kind="ExternalInput")      # upd_b1+b2@U1b
    b2uc = nc.dram_tensor("b2uc", [HID, 1], f32, kind="ExternalInput")    # upd_b2 col
    lnbc = nc.dram_tensor("lnbc", [HID, 1], f32, kind="ExternalInput")    # ln_b col
    g_row = nc.dram_tensor("g_row", [1, HID], DT, kind="ExternalInput")   # ln_g
    fixf = nc.dram_tensor("fixf", [1, K], f32, kind="ExternalInput")      # 12/count head
    fixl = nc.dram_tensor("fixl", [1, K], f32, kind="ExternalInput")      # 12/count tail
    # band-select matrix: column 63 = 1/128, else 0 (stats row packing)
    selb = nc.dram_tensor("selb", [HID, 2 * 2 * NCH - 1], DT, kind="ExternalInput")
    outT = nc.dram_tensor("outT", [HID, N], DT, kind="ExternalOutput")

    Silu = mybir.ActivationFunctionType.Silu
    Sqrt = mybir.ActivationFunctionType.Sqrt
    HOT = 2 * NCH - 1   # hot column index in selb

    with tile.TileContext(nc) as tc:
        with (
            tc.tile_pool(name="singles", bufs=1) as singles,
            tc.tile_pool(name="big", bufs=1) as big,
            tc.tile_pool(name="work", bufs=3) as work,
            tc.tile_pool(name="zpool", bufs=3) as zpool,
            tc.tile_pool(name="opool", bufs=3) as opool,
            tc.tile_pool(name="stage", bufs=3) as stpool,
            tc.tile_pool(name="psDE", bufs=2, space="PSUM") as psDE,
            tc.tile_pool(name="psUX", bufs=3, space="PSUM") as psUX,
            tc.tile_pool(name="psPP", bufs=2, space="PSUM") as psPP,
            tc.tile_pool(name="psS", bufs=1, space="PSUM") as psS,
        ):
            # ---- constants into SBUF ----
            sW1a = singles.tile([HID, HID], DT)
            sW1b = singles.tile([HID, HID], DT)
            sW2s = singles.tile([HID, HID], DT)
            sW2u = singles.tile([HID, HID], DT)
            sU1a = singles.tile([HID, HID], DT)
            sU1b = singles.tile([HID, HID], DT)
            sU2 = singles.tile([HID, HID], DT)
            sIdent = singles.tile([HID, HID], DT)
            sw1c = singles.tile([1, HID], DT)
            sw1cn = singles.tile([1, HID], DT)
            sg = singles.tile([1, HID], DT)
            # D/E-phase weights first, spread across queues so the
            # pipeline head isn't gated by one queue's serial issue rate
            nc.scalar.dma_start(out=sW1b, in_=W1b[:, :])
            nc.gpsimd.dma_start(out=sw1c, in_=w1c[:, :])
            nc.scalar.dma_start(out=sW1a, in_=W1a[:, :])
            nc.sync.dma_start(out=sw1cn, in_=w1cn[:, :])
            sb1 = singles.tile([HID, 1], f32)
            sbu = singles.tile([HID, 1], f32)
            sb2u = singles.tile([HID, 1], f32)
            slnb = singles.tile([HID, 1], f32)
            nc.gpsimd.dma_start(out=sb1, in_=b1c[:, :])
            for sb, dr in [(sW2u, W2u), (sW2s, W2s), (sU1a, U1a), (sU1b, U1b),
                           (sU2, U2), (sIdent, ident), (sg, g_row)]:
                nc.scalar.dma_start(out=sb, in_=dr[:, :])
            for sb, dr in [(sbu, buc), (sb2u, b2uc), (slnb, lnbc)]:
                nc.gpsimd.dma_start(out=sb, in_=dr[:, :])
            # broadcast [1,6] -> [128,6] fix tiles
            sfixf = singles.tile([HID, K], f32)
            sfixl = singles.tile([HID, K], f32)

            def bcast_rows(dr):
                a = dr[0:1, :]
                return bass.AP(tensor=a.tensor, offset=a.offset,
                               ap=[[0, HID]] + list(a.ap[1:]))

            nc.gpsimd.dma_start(out=sfixf, in_=bcast_rows(fixf))
            nc.gpsimd.dma_start(out=sfixl, in_=bcast_rows(fixl))
            ssel = singles.tile([HID, 2 * 2 * NCH - 1], DT)
            nc.scalar.dma_start(out=ssel, in_=selb[:, :])

            # ---- big persistent buffers ----
            D_A = big.tile([HID, NCOL], DT)      # token j at col OFF0 + j
            D_B = big.tile([HID, NCOL], DT)      # token j at col OFF0 + 1 + j
            x_full = big.tile([HID, N], DT)
            # zero halo columns of D so boundary silu stays finite
            nc.vector.memset(D_A[:, 0:OFF0], 0.0)
            nc.vector.memset(D_A[:, OFF0 + N:NCOL], 0.0)
            nc.vector.memset(D_B[:, 0:OFF0 + 1], 0.0)
            nc.vector.memset(D_B[:, OFF0 + 1 + N:NCOL], 0.0)

            # LN stats: ONE PSUM bank, quadrant layout (all DVE reads start at
            # a 32-aligned partition): half h: E[x] rows 64h+0:16,
            # E[x2] rows 64h+32:48.
            st_ps = psS.tile([128, T], f32, tag="st")

            # r|u rows for the normalize pass: row i = [r (T) | mu*r (T)]
            # (one tile per half so DVE writes start at partition 0)
            ru_sb0 = big.tile([NHALF, 2 * T], DT)
            ru_sb1 = big.tile([NHALF, 2 * T], DT)
            ru31 = big.tile([1, 2 * T], DT)
            seps = singles.tile([NHALF, 1], f32)
            nc.vector.memset(seps, float(EPS))
            scr = singles.tile([1, 4], f32)
            nc.vector.memset(scr, 1.0)

            hts = {}
            crd = {}
            zs = {}

            def load_chunk(c):
                # ht lives from load (iter c-2) to the x-op (iter c+2)
                ht = work.tile([HID, T], DT, tag="ht", bufs=6)
                nc.sync.dma_start(out=ht, in_=hT[:, c * T:(c + 1) * T])
                co = work.tile([1, T], DT, tag="co", bufs=4)
                nc.sync.dma_start(out=co, in_=coordR[:, c * T:(c + 1) * T])
                hts[c] = ht
                crd[c] = co

            dps = {}
            eps = {}

            def phase_de_mm(c):
                # D/E chunk matmuls (iteration front: keeps the PE fed)
                d_ps = psDE.tile([HID, T], f32, tag="de")
                nc.tensor.matmul(d_ps, sW1b, hts[c], start=True, stop=False)
                nc.tensor.matmul(d_ps, sw1c, crd[c], start=False, stop=True)
                e_ps = psDE.tile([HID, T], f32, tag="de")
                nc.tensor.matmul(e_ps, sW1a, hts[c], start=True, stop=False)
                nc.tensor.matmul(e_ps, sw1cn, crd[c], start=False, stop=True)
                dps[c] = d_ps
                eps[c] = e_ps

            def phase_de_cast(c):
                # casts queue at DVE tail so the z-builds keep priority
                col = OFF0 + c * T
                nc.vector.tensor_copy(D_A[:, col:col + T], dps.pop(c))
                # shifted copy for odd-offset alignment: SBUF->SBUF DMA
                nc.sync.dma_start(out=D_B[:, col + 1:col + 1 + T],
                                  in_=D_A[:, col:col + T])
                e_sb = work.tile([HID, T], DT, tag="esb")
                nc.vector.tensor_copy(e_sb, eps.pop(c))
                esbs[c] = e_sb

            def seg_in1(tile_ap, col, n):
                # [128, n, T] AP over D with outer column-stride 2
                s = tile_ap[:, col:col + T]
                return bass.AP(tensor=s.tensor, offset=s.offset,
                               ap=[s.ap[0], [2, n], [1, T]])

            def e_bcast(e_sb, n):
                return bass.AP(tensor=e_sb.tensor, offset=e_sb.offset,
                               ap=[e_sb.ap[0], [0, n], [1, T]])

            esbs = {}
            aps = {}
            us = {}
            s2s = {}

            def zbuild(t):
                # build all 12 segments in two DVE ops, then one 12T silu.
                # D_B covers odd offsets -5..-1,+1..+5 = uniform stride 2;
                # D_A covers -6,-4,-2 and +2,+4,+6 = two stride-2 triples
                # with an outer jump of 8 columns (4D access pattern).
                e_sb = esbs.pop(t)
                z = zpool.tile([HID, 12 * T], DT, tag="z")
                zv = z.rearrange("p (s t) -> p s t", t=T)
                base = t * T
                # segs 0-5 <- D_A even offsets -6,-4,-2,+2,+4,+6 (4D in1: two
                # stride-2 triples with an outer jump of 8 columns)
                da = D_A[:, OFF0 + base - 6:OFF0 + base - 6 + T]
                in1_4d = bass.AP(tensor=da.tensor, offset=da.offset,
                                 ap=[da.ap[0], [8, 2], [2, 3], [1, T]])
                in0_4d = bass.AP(tensor=e_sb.tensor, offset=e_sb.offset,
                                 ap=[e_sb.ap[0], [0, 2], [0, 3], [1, T]])
                nc.vector.tensor_tensor(out=zv[:, 0:6, :], in0=in0_4d,
                                        in1=in1_4d, op=mybir.AluOpType.add)
                # silu A-block immediately: its matmuls release early
                nc.scalar.activation(z[:, 0:6 * T], z[:, 0:6 * T], Silu,
                                     bias=sb1, scale=1.0)
                # segs 6-11 <- D_B odd offsets (uniform stride 2, one 3D op)
                nc.vector.tensor_tensor(
                    out=zv[:, 6:12, :], in0=e_bcast(e_sb, 6),
                    in1=seg_in1(D_B, OFF0 + 1 + base - 5, 6),
                    op=mybir.AluOpType.add)
                nc.scalar.activation(z[:, 6 * T:12 * T], z[:, 6 * T:12 * T],
                                     Silu, bias=sb1, scale=1.0)
                zs[t] = (z, zv)

            def msgAll(t):
                # all 12 message matmuls (+U1a for interior) in one burst
                _, zv = zs.pop(t)
                boundary = t == 0 or t == NCH - 1
                if t == 0:
                    for s, d in enumerate(SEG_ORDER):
                        if d < 0:
                            nc.vector.memset(zv[:, s, 0:-d], 0.0)
                if t == NCH - 1:
                    for s, d in enumerate(SEG_ORDER):
                        if d > 0:
                            nc.vector.memset(zv[:, s, T - d:T], 0.0)
                if boundary:
                    a_ps = psUX.tile([HID, T], f32, tag="ux")
                    for s in range(12):
                        nc.tensor.matmul(a_ps, sW2s, zv[:, s, :],
                                         start=(s == 0), stop=(s == 11))
                    aps[t] = a_ps
                else:
                    u_ps = psUX.tile([HID, T], f32, tag="ux")
                    nc.tensor.matmul(u_ps, sU1a, hts[t], start=True, stop=False)
                    for s in range(12):
                        nc.tensor.matmul(u_ps, sW2u, zv[:, s, :],
                                         start=False, stop=(s == 11))
                    us[t] = u_ps

            def s2em(t):
                # interior: silu of update-MLP hidden
                s2 = work.tile([HID, T], DT, tag="s2")
                nc.scalar.activation(s2, us.pop(t), Silu, bias=sbu, scale=1.0)
                s2s[t] = s2

            def bfix(t):
                # boundary chunks: explicit agg + count fixup + U1b path
                a_ps = aps.pop(t)
                agg = work.tile([HID, T], DT, tag="agg_sb")
                nc.vector.tensor_copy(agg, a_ps)
                if t == 0:
                    nc.vector.tensor_tensor(
                        out=agg[:, 0:K], in0=a_ps[:, 0:K],
                        in1=sfixf, op=mybir.AluOpType.mult)
                else:
                    nc.vector.tensor_tensor(
                        out=agg[:, T - K:T], in0=a_ps[:, T - K:T],
                        in1=sfixl, op=mybir.AluOpType.mult)
                u_ps = psUX.tile([HID, T], f32, tag="ux")
                nc.tensor.matmul(u_ps, sU1a, hts[t], start=True, stop=False)
                nc.tensor.matmul(u_ps, sU1b, agg, start=False, stop=True)
                s2 = work.tile([HID, T], DT, tag="s2")
                nc.scalar.activation(s2, u_ps, Silu, bias=sbu, scale=1.0)
                s2s[t] = s2

            xps = {}
            x2s = {}

            def tailA(t):
                # x_psum = U2.T @ s2 + Ident @ h
                x_ps = psUX.tile([HID, T], f32, tag="ux")
                nc.tensor.matmul(x_ps, sU2, s2s.pop(t), start=True, stop=False)
                nc.tensor.matmul(x_ps, sIdent, hts.pop(t), start=False, stop=True)
                xps[t] = x_ps

            def tailB_dve(t):
                # x = x_psum + b2u  (plain cast with per-partition bias)
                x_ps = xps.pop(t)
                base = t * T
                x_sb = x_full[:, base:base + T]
                nc.vector.tensor_scalar(out=x_sb, in0=x_ps, scalar1=sb2u,
                                        scalar2=None, op0=mybir.AluOpType.add)
                x2 = work.tile([HID, T], DT, tag="x2")
                nc.vector.tensor_tensor(out=x2, in0=x_sb, in1=x_sb,
                                        op=mybir.AluOpType.mult)
                x2s[t] = x2

            def stats_pe(t):
                # stats rows (quadrant bank): E[x] row i, E[x2] row 32+i of the
                # group's 64-row window.  Groups: chunks 0-15 -> window 0,
                # 16-30 -> window 64 (batched right after chunk 30), 31 ->
                # window 0 reused (its own tiny group).
                base = t * T
                x_sb = x_full[:, base:base + T]
                x2 = x2s.pop(t)
                h_, i_ = t // NHALF, t % NHALF
                first = i_ == 0
                last = i_ == NHALF - 1
                st = st_ps[64 * h_:64 * h_ + 64, :]
                r_e2 = 2 * NHALF + i_
                nc.tensor.matmul(st, ssel[:, HOT - i_:HOT - i_ + 4 * NHALF],
                                 x_sb, start=first, stop=False)
                nc.tensor.matmul(st, ssel[:, HOT - r_e2:HOT - r_e2 + 4 * NHALF],
                                 x2, start=False, stop=last)

            smstash = {}

            def stats_mathA(r0, n):
                # first half of the batch math: moments -> sd
                ex_sb = work.tile([NHALF, T], f32, tag="ex")
                nc.vector.tensor_copy(ex_sb[0:n, :], st_ps[r0:r0 + n, :])
                t1 = work.tile([NHALF, T], f32, tag="t1")
                nc.vector.tensor_tensor(out=t1[0:n, :], in0=ex_sb[0:n, :],
                                        in1=ex_sb[0:n, :],
                                        op=mybir.AluOpType.mult)
                var = work.tile([NHALF, T], f32, tag="var")
                nc.vector.tensor_tensor(
                    out=var[0:n, :], in0=st_ps[r0 + 32:r0 + 32 + n, :],
                    in1=t1[0:n, :], op=mybir.AluOpType.subtract)
                nc.scalar.activation(var[0:n, :], var[0:n, :], Sqrt,
                                     bias=seps[0:n], scale=1.0)
                smstash[r0] = (ex_sb, var)

            def stats_mathB(r0, n, ru):
                # second half: reciprocal + mu*r rows
                ex_sb, var = smstash.pop(r0)
                with nc.allow_low_precision(reason="rstd rows feed fp16 matmuls"):
                    nc.vector.reciprocal(out=ru[0:n, 0:T], in_=var[0:n, :])
                nc.vector.tensor_tensor(out=ru[0:n, T:2 * T], in0=ex_sb[0:n, :],
                                        in1=ru[0:n, 0:T],
                                        op=mybir.AluOpType.mult)

            def stats_math(r0, n, ru):
                # batched per-token LN stats for n chunks at bank window r0
                # E[x] rows to SBUF; E[x2] stays in PSUM (rows r0+32.. —
                # 32-aligned; PSUM+SB operand bases may differ, SB+SB may not)
                ex_sb = work.tile([NHALF, T], f32, tag="ex")
                nc.vector.tensor_copy(ex_sb[0:n, :], st_ps[r0:r0 + n, :])
                t1 = work.tile([NHALF, T], f32, tag="t1")
                nc.vector.tensor_tensor(out=t1[0:n, :], in0=ex_sb[0:n, :],
                                        in1=ex_sb[0:n, :],
                                        op=mybir.AluOpType.mult)
                var = work.tile([NHALF, T], f32, tag="var")
                nc.vector.tensor_tensor(
                    out=var[0:n, :], in0=st_ps[r0 + 32:r0 + 32 + n, :],
                    in1=t1[0:n, :], op=mybir.AluOpType.subtract)
                nc.scalar.activation(var[0:n, :], var[0:n, :], Sqrt,
                                     bias=seps[0:n], scale=1.0)
                with nc.allow_low_precision(reason="rstd rows feed fp16 matmuls"):
                    nc.vector.reciprocal(out=ru[0:n, 0:T], in_=var[0:n, :])
                nc.vector.tensor_tensor(out=ru[0:n, T:2 * T], in0=ex_sb[0:n, :],
                                        in1=ru[0:n, 0:T],
                                        op=mybir.AluOpType.mult)

            p2live = {}

            def pass2_pe(t, pool=None):
                # normalize chunk t, matmul part: p1 = g x r, p2 = g x mu*r
                ru = stpool.tile([1, 2 * T], DT, tag="ru")
                src = ru_sb0 if t < NHALF else ru_sb1
                nc.gpsimd.dma_start(out=ru, in_=src[t % NHALF:t % NHALF + 1, :])
                pool, tg = (psPP, "pp") if pool is None else (psUX, "ux")
                p1 = pool.tile([HID, T], f32, tag=tg)
                nc.tensor.matmul(p1, sg, ru[0:1, 0:T], start=True, stop=True)
                p2 = pool.tile([HID, T], f32, tag=tg)
                nc.tensor.matmul(p2, sg, ru[0:1, T:2 * T], start=True, stop=True)
                p2live[t] = (p1, p2)

            def pass2_dve(t):
                # out = x*p1 + lnb - p2
                base = t * T
                p1, p2 = p2live.pop(t)
                o = opool.tile([HID, T], DT, tag="o")
                nc.vector.tensor_tensor(out=o, in0=x_full[:, base:base + T],
                                        in1=p1, op=mybir.AluOpType.mult)
                nc.vector.scalar_tensor_tensor(
                    out=o, in0=o, scalar=slnb, in1=p2,
                    op0=mybir.AluOpType.add, op1=mybir.AluOpType.subtract)
                if t % 2 == 0:
                    nc.scalar.dma_start(out=outT[:, base:base + T], in_=o)
                else:
                    nc.sync.dma_start(out=outT[:, base:base + T], in_=o)

            # ---------------- fused pipeline ----------------
            # D/E run THREE chunks ahead (emitted at iteration end), so at
            # every iteration start each engine's queue head is ready:
            #   PE : U2+Id(c-2) | stats(c-3) | msgAll(c-1) | p1,p2 | D,E(c+2)
            #   DVE: build(c) x2 | x(c-2), x2(c-2) | o1, o2 | casts(c+2)
            #   ACT: silu(c) | s2(c-1)
            p2q = []
            for t in range(3):
                load_chunk(t)
            phase_de_mm(0)
            phase_de_cast(0)
            phase_de_mm(1)
            phase_de_cast(1)
            for c in range(NCH + 3):
                if c + 2 < NCH:
                    phase_de_mm(c + 2)
                if 2 <= c <= NCH + 1 and (c - 2) in s2s:
                    tailA(c - 2)
                if 3 <= c and (c - 3) in x2s:
                    stats_pe(c - 3)
                if 1 <= c <= NCH:
                    msgAll(c - 1)
                pj = p2q.pop(0) if (p2q and c >= NHALF + 3) else None
                if c < NCH:
                    zbuild(c)
                if c + 2 < NCH:
                    phase_de_cast(c + 2)
                if 2 <= c <= NCH + 1 and (c - 2) in xps:
                    tailB_dve(c - 2)
                if 1 <= c <= NCH:
                    t = c - 1
                    if t == 0 or t == NCH - 1:
                        bfix(t)
                    else:
                        s2em(t)
                if c == NCH:
                    # compress the chunk-31 wind-down: finish its tail chain
                    # and the half-1 batch math ahead of the remaining
                    # in-loop pass2 jobs, releasing the tail drain early
                    tailA(NCH - 1)
                    tailB_dve(NCH - 1)
                    stats_pe(NCH - 2)
                    stats_pe(NCH - 1)
                    nc.scalar.activation(scr, scr, Sqrt, bias=0.0, scale=1.0)
                    stats_math(64, NHALF, ru_sb1)
                    p2q.extend(range(NHALF, NCH))
                if c - 3 == NHALF - 1:
                    stats_math(0, NHALF, ru_sb0)
                    p2q.extend(range(NHALF))
                if c + 3 < NCH:
                    load_chunk(c + 3)
                if pj is not None:
                    pass2_pe(pj)
                    pass2_dve(pj)
            rest = list(p2q)
            pass2_pe(rest[0])
            for i, t in enumerate(rest):
                if i + 1 < len(rest):
                    # ux banks are free in the tail: alternate pools for
                    # deeper grid-matmul pipelining
                    pass2_pe(rest[i + 1], psUX if (i % 2 == 0) else None)
                pass2_dve(t)

    nc.compile()
    return nc


def _get_compiled(dt_name):
    global _compiled
    if _compiled is None:
        from concourse import mybir
        dt = {"bf16": mybir.dt.bfloat16, "fp16": mybir.dt.float16,
              "fp32": mybir.dt.float32}[dt_name]
        _compiled = _build_bass(dt)
    return _compiled


DT_NAME = "fp16"


def _sel_band(act_np):
    sel = np.zeros((HID, 2 * 2 * NCH - 1), dtype=np.float32)
    sel[:, 2 * NCH - 1] = 1.0 / HID
    return sel.astype(act_np)


def kernel(**inputs):
    from concourse.bass_utils import run_bass_kernel_spmd

    h = np.asarray(inputs["h"], dtype=np.float32)
    coord = np.asarray(inputs["coord"], dtype=np.float32)
    msg_w1 = np.asarray(inputs["msg_w1"], dtype=np.float32)
    msg_b1 = np.asarray(inputs["msg_b1"], dtype=np.float32)
    msg_w2 = np.asarray(inputs["msg_w2"], dtype=np.float32)
    msg_b2 = np.asarray(inputs["msg_b2"], dtype=np.float32)
    upd_w1 = np.asarray(inputs["upd_w1"], dtype=np.float32)
    upd_b1 = np.asarray(inputs["upd_b1"], dtype=np.float32)
    upd_w2 = np.asarray(inputs["upd_w2"], dtype=np.float32)
    upd_b2 = np.asarray(inputs["upd_b2"], dtype=np.float32)
    ln_g = np.asarray(inputs["ln_g"], dtype=np.float32)
    ln_b = np.asarray(inputs["ln_b"], dtype=np.float32)

    import ml_dtypes
    act_np = {"bf16": ml_dtypes.bfloat16, "fp16": np.float16,
              "fp32": np.float32}[DT_NAME]

    W1a = msg_w1[:HID]
    W1b = msg_w1[HID:2 * HID]
    w1c = msg_w1[2 * HID]
    U1b_f = upd_w1[HID:2 * HID]
    bias_u = upd_b1 + msg_b2 @ U1b_f
    W2s = msg_w2 / (2.0 * K)
    W2u = W2s @ U1b_f

    idx = np.arange(N)
    count = (np.minimum(idx, K) + np.minimum(N - 1 - idx, K)).astype(np.float32)
    fix = (2.0 * K) / count
    fixf = fix[:K].reshape(1, K).astype(np.float32)
    fixl = fix[N - K:].reshape(1, K).astype(np.float32)

    const = {
        "W1a": np.ascontiguousarray(W1a, dtype=act_np),
        "W1b": np.ascontiguousarray(W1b, dtype=act_np),
        "w1c": np.ascontiguousarray(w1c.reshape(1, HID), dtype=act_np),
        "w1cn": np.ascontiguousarray(-w1c.reshape(1, HID), dtype=act_np),
        "W2s": np.ascontiguousarray(W2s, dtype=act_np),
        "W2u": np.ascontiguousarray(W2u, dtype=act_np),
        "U1a": np.ascontiguousarray(upd_w1[:HID], dtype=act_np),
        "U1b": np.ascontiguousarray(U1b_f, dtype=act_np),
        "U2": np.ascontiguousarray(upd_w2, dtype=act_np),
        "b1c": np.ascontiguousarray(msg_b1.reshape(HID, 1), dtype=np.float32),
        "buc": np.ascontiguousarray(bias_u.reshape(HID, 1), dtype=np.float32),
        "b2uc": np.ascontiguousarray(upd_b2.reshape(HID, 1), dtype=np.float32),
        "lnbc": np.ascontiguousarray(ln_b.reshape(HID, 1), dtype=np.float32),
        "g_row": np.ascontiguousarray(ln_g.reshape(1, HID), dtype=act_np),
        "ident": np.ascontiguousarray(np.eye(HID), dtype=act_np),
        "fixf": fixf,
        "fixl": fixl,
        "selb": _sel_band(act_np),
    }

    in_maps = []
    for b in range(B):
        m = dict(const)
        m["hT"] = np.ascontiguousarray(h[b].T, dtype=act_np)
        m["coordR"] = np.ascontiguousarray(coord[b].reshape(1, N), dtype=act_np)
        in_maps.append(m)

    nc = _get_compiled(DT_NAME)
    res = run_bass_kernel_spmd(nc, in_maps, core_ids=list(range(B)))
    global LAST_RESULTS
    LAST_RESULTS = res
    out = np.stack([np.asarray(res.results[b]["outT"], dtype=np.float32).T
                    for b in range(B)])
    return np.ascontiguousarray(out)


# revision 59
# speedup vs baseline: 1.0246x; 1.0246x over previous
"""Trainium2 Bass kernel for the LocalGNOBlock (windowed GNN message passing).

Math restructuring (vs the naive 12x full MLP evaluations):
  msg first layer is linear over concat([h_i, h_j, dc]):
      z_d[i] = (A - C)[i] + (B + C)[i+d] + b1,  d in {+-1..+-6}
  where A = h @ W1a, B = h @ W1b, C = coord x w1c (rank-1).
  Interior chunks fold the whole message-2nd-layer + U1b product:
      u += sum_d silu(z_d) @ (W2/12 @ U1b)     (12 matmuls, PSUM accum)
  so the "agg" tensor is never materialized except at the two boundary
  chunks (count fixup).  LayerNorm stats are per-token (channel dim on
  partitions) via band-select ones matmuls packed into one PSUM bank in
  two half-batches, so normalization of the first half overlaps pass-1
  compute of the second half.

Engine budget per 512-token chunk (targets):
  ACT   silu(12T) 5.4us + silu(s2) 0.7us            -> floor ~6.1us
  DVE   z-build 3.4 + E/D casts 1.4 + x 0.7 + norm  -> ~5.5-6.9us
  PE    ~22 matmuls x 215ns (warm clock)            -> ~5us
  GPSIMD x^2 (SBUF only - no PSUM port)             -> ~1.9us
  D_B shifted copy runs as SBUF->SBUF DMA.

Sharding: batch dim B=8 -> one batch element per NeuronCore (no halo).
Host pre/post: transpose h -> [128, N] per core, transpose back after.
"""

import numpy as np

K = 6
HID = 128
N = 16384
B = 8
EPS = 1e-5
T = 512                 # token chunk (matmul + elementwise granularity)
NCH = N // T            # 32 chunks
NHALF = NCH // 2        # stats half-batch
OFF0 = 8                # D_full column of token 0 (even, for fp16 alignment)
NCOL = N + 2 * OFF0     # D_full width

# z segment order: A-block = D_A even offsets (segs 0-5), B-block = D_B odd
# offsets (segs 6-11); each block is one DVE build op + one silu op, so the
# A-block message matmuls release as soon as silu-A completes.
SEG_ORDER = [-6, -4, -2, 2, 4, 6, -5, -3, -1, 1, 3, 5]

_compiled = None


def _build_bass(dt_act):
    import concourse.bacc as bacc
    import concourse.bass as bass
    import concourse.tile as tile
    from concourse import mybir

    f32 = mybir.dt.float32
    DT = dt_act

    nc = bacc.Bacc("TRN2", target_bir_lowering=False, debug=False)

    # ---- DRAM I/O ----
    hT = nc.dram_tensor("hT", [HID, N], DT, kind="ExternalInput")
    coordR = nc.dram_tensor("coordR", [1, N], DT, kind="ExternalInput")
    W1a = nc.dram_tensor("W1a", [HID, HID], DT, kind="ExternalInput")
    W1b = nc.dram_tensor("W1b", [HID, HID], DT, kind="ExternalInput")
    w1c = nc.dram_tensor("w1c", [1, HID], DT, kind="ExternalInput")      # +w1c
    w1cn = nc.dram_tensor("w1cn", [1, HID], DT, kind="ExternalInput")    # -w1c
    W2s = nc.dram_tensor("W2s", [HID, HID], DT, kind="ExternalInput")    # W2/12
    W2u = nc.dram_tensor("W2u", [HID, HID], DT, kind="ExternalInput")    # W2/12@U1b
    U1a = nc.dram_tensor("U1a", [HID, HID], DT, kind="ExternalInput")
    U1b = nc.dram_tensor("U1b", [HID, HID], DT, kind="ExternalInput")
    U2 = nc.dram_tensor("U2", [HID, HID], DT, kind="ExternalInput")
    ident = nc.dram_tensor("ident", [HID, HID], DT, kind="ExternalInput")
    b1c = nc.dram_tensor("b1c", [HID, 1], f32, kind="ExternalInput")      # msg_b1
    buc = nc.dram_tensor("buc", [HID, 1], f32, kind="ExternalInput")      # upd_b1+b2@U1b
    b2uc = nc.dram_tensor("b2uc", [HID, 1], f32, kind="ExternalInput")    # upd_b2 col
    lnbc = nc.dram_tensor("lnbc", [HID, 1], f32, kind="ExternalInput")    # ln_b col
    g_row = nc.dram_tensor("g_row", [1, HID], DT, kind="ExternalInput")   # ln_g
    fixf = nc.dram_tensor("fixf", [1, K], f32, kind="ExternalInput")      # 12/count head
    fixl = nc.dram_tensor("fixl", [1, K], f32, kind="ExternalInput")      # 12/count tail
    # band-select matrix: column 63 = 1/128, else 0 (stats row packing)
    selb = nc.dram_tensor("selb", [HID, 2 * 2 * NCH - 1], DT, kind="ExternalInput")
    outT = nc.dram_tensor("outT", [HID, N], DT, kind="ExternalOutput")

    Silu = mybir.ActivationFunctionType.Silu
    Sqrt = mybir.ActivationFunctionType.Sqrt
    HOT = 2 * NCH - 1   # hot column index in selb

    with tile.TileContext(nc) as tc:
        with (
            tc.tile_pool(name="singles", bufs=1) as singles,
            tc.tile_pool(name="big", bufs=1) as big,
            tc.tile_pool(name="work", bufs=3) as work,
            tc.tile_pool(name="zpool", bufs=3) as zpool,
            tc.tile_pool(name="opool", bufs=3) as opool,
            tc.tile_pool(name="stage", bufs=3) as stpool,
            tc.tile_pool(name="psDE", bufs=2, space="PSUM") as psDE,
            tc.tile_pool(name="psUX", bufs=3, space="PSUM") as psUX,
            tc.tile_pool(name="psPP", bufs=2, space="PSUM") as psPP,
            tc.tile_pool(name="psS", bufs=1, space="PSUM") as psS,
        ):
            # ---- constants into SBUF ----
            sW1a = singles.tile([HID, HID], DT)
            sW1b = singles.tile([HID, HID], DT)
            sW2s = singles.tile([HID, HID], DT)
            sW2u = singles.tile([HID, HID], DT)
            sU1a = singles.tile([HID, HID], DT)
            sU1b = singles.tile([HID, HID], DT)
            sU2 = singles.tile([HID, HID], DT)
            sIdent = singles.tile([HID, HID], DT)
            sw1c = singles.tile([1, HID], DT)
            sw1cn = singles.tile([1, HID], DT)
            sg = singles.tile([1, HID], DT)
            # D/E-phase weights first, spread across queues so the
            # pipeline head isn't gated by one queue's serial issue rate
            nc.scalar.dma_start(out=sW1b, in_=W1b[:, :])
            nc.gpsimd.dma_start(out=sw1c, in_=w1c[:, :])
            nc.scalar.dma_start(out=sW1a, in_=W1a[:, :])
            nc.sync.dma_start(out=sw1cn, in_=w1cn[:, :])
            sb1 = singles.tile([HID, 1], f32)
            sbu = singles.tile([HID, 1], f32)
            sb2u = singles.tile([HID, 1], f32)
            slnb = singles.tile([HID, 1], f32)
            nc.gpsimd.dma_start(out=sb1, in_=b1c[:, :])
            for sb, dr in [(sW2u, W2u), (sW2s, W2s), (sU1a, U1a), (sU1b, U1b),
                           (sU2, U2), (sIdent, ident), (sg, g_row)]:
                nc.scalar.dma_start(out=sb, in_=dr[:, :])
            for sb, dr in [(sbu, buc), (sb2u, b2uc), (slnb, lnbc)]:
                nc.gpsimd.dma_start(out=sb, in_=dr[:, :])
            # broadcast [1,6] -> [128,6] fix tiles
            sfixf = singles.tile([HID, K], f32)
            sfixl = singles.tile([HID, K], f32)

            def bcast_rows(dr):
                a = dr[0:1, :]
                return bass.AP(tensor=a.tensor, offset=a.offset,
                               ap=[[0, HID]] + list(a.ap[1:]))

            nc.gpsimd.dma_start(out=sfixf, in_=bcast_rows(fixf))
            nc.gpsimd.dma_start(out=sfixl, in_=bcast_rows(fixl))
            ssel = singles.tile([HID, 2 * 2 * NCH - 1], DT)
            nc.scalar.dma_start(out=ssel, in_=selb[:, :])

            # ---- big persistent buffers ----
            D_A = big.tile([HID, NCOL], DT)      # token j at col OFF0 + j
            D_B = big.tile([HID, NCOL], DT)      # token j at col OFF0 + 1 + j
            x_full = big.tile([HID, N], DT)
            # zero halo columns of D so boundary silu stays finite
            nc.vector.memset(D_A[:, 0:OFF0], 0.0)
            nc.vector.memset(D_A[:, OFF0 + N:NCOL], 0.0)
            nc.vector.memset(D_B[:, 0:OFF0 + 1], 0.0)
            nc.vector.memset(D_B[:, OFF0 + 1 + N:NCOL], 0.0)

            # LN stats: ONE PSUM bank, quadrant layout (all DVE reads start at
            # a 32-aligned partition): half h: E[x] rows 64h+0:16,
            # E[x2] rows 64h+32:48.
            st_ps = psS.tile([128, T], f32, tag="st")

            # r|u rows for the normalize pass: row i = [r (T) | mu*r (T)]
            # (one tile per half so DVE writes start at partition 0)
            ru_sb0 = big.tile([NHALF, 2 * T], DT)
            ru_sb1 = big.tile([NHALF, 2 * T], DT)
            ru31 = big.tile([1, 2 * T], DT)
            seps = singles.tile([NHALF, 1], f32)
            nc.vector.memset(seps, float(EPS))
            scr = singles.tile([1, 4], f32)
            nc.vector.memset(scr, 1.0)

            hts = {}
            crd = {}
            zs = {}

            def load_chunk(c):
                # ht lives from load (iter c-2) to the x-op (iter c+2)
                ht = work.tile([HID, T], DT, tag="ht", bufs=6)
                nc.sync.dma_start(out=ht, in_=hT[:, c * T:(c + 1) * T])
                co = work.tile([1, T], DT, tag="co", bufs=4)
                nc.sync.dma_start(out=co, in_=coordR[:, c * T:(c + 1) * T])
                hts[c] = ht
                crd[c] = co

            dps = {}
            eps = {}

            def phase_de_mm(c):
                # D/E chunk matmuls (iteration front: keeps the PE fed)
                d_ps = psDE.tile([HID, T], f32, tag="de")
                nc.tensor.matmul(d_ps, sW1b, hts[c], start=True, stop=False)
                nc.tensor.matmul(d_ps, sw1c, crd[c], start=False, stop=True)
                e_ps = psDE.tile([HID, T], f32, tag="de")
                nc.tensor.matmul(e_ps, sW1a, hts[c], start=True, stop=False)
                nc.tensor.matmul(e_ps, sw1cn, crd[c], start=False, stop=True)
                dps[c] = d_ps
                eps[c] = e_ps

            def phase_de_cast(c):
                # casts queue at DVE tail so the z-builds keep priority
                col = OFF0 + c * T
                nc.vector.tensor_copy(D_A[:, col:col + T], dps.pop(c))
                # shifted copy for odd-offset alignment: SBUF->SBUF DMA
                nc.sync.dma_start(out=D_B[:, col + 1:col + 1 + T],
                                  in_=D_A[:, col:col + T])
                e_sb = work.tile([HID, T], DT, tag="esb")
                nc.vector.tensor_copy(e_sb, eps.pop(c))
                esbs[c] = e_sb

            def seg_in1(tile_ap, col, n):
                # [128, n, T] AP over D with outer column-stride 2
                s = tile_ap[:, col:col + T]
                return bass.AP(tensor=s.tensor, offset=s.offset,
                               ap=[s.ap[0], [2, n], [1, T]])

            def e_bcast(e_sb, n):
                return bass.AP(tensor=e_sb.tensor, offset=e_sb.offset,
                               ap=[e_sb.ap[0], [0, n], [1, T]])

            esbs = {}
            aps = {}
            us = {}
            s2s = {}

            def zbuild(t):
                # build all 12 segments in two DVE ops, then one 12T silu.
                # D_B covers odd offsets -5..-1,+1..+5 = uniform stride 2;
                # D_A covers -6,-4,-2 and +2,+4,+6 = two stride-2 triples
                # with an outer jump of 8 columns (4D access pattern).
                e_sb = esbs.pop(t)
                z = zpool.tile([HID, 12 * T], DT, tag="z")
                zv = z.rearrange("p (s t) -> p s t", t=T)
                base = t * T
                # segs 0-5 <- D_A even offsets -6,-4,-2,+2,+4,+6 (4D in1: two
                # stride-2 triples with an outer jump of 8 columns)
                da = D_A[:, OFF0 + base - 6:OFF0 + base - 6 + T]
                in1_4d = bass.AP(tensor=da.tensor, offset=da.offset,
                                 ap=[da.ap[0], [8, 2], [2, 3], [1, T]])
                in0_4d = bass.AP(tensor=e_sb.tensor, offset=e_sb.offset,
                                 ap=[e_sb.ap[0], [0, 2], [0, 3], [1, T]])
                nc.vector.tensor_tensor(out=zv[:, 0:6, :], in0=in0_4d,
                                        in1=in1_4d, op=mybir.AluOpType.add)
                # silu A-block immediately: its matmuls release early
                nc.scalar.activation(z[:, 0:6 * T], z[:, 0:6 * T], Silu,
                                     bias=sb1, scale=1.0)
                # segs 6-11 <- D_B odd offsets (uniform stride 2, one 3D op)
                nc.vector.tensor_tensor(
                    out=zv[:, 6:12, :], in0=e_bcast(e_sb, 6),
                    in1=seg_in1(D_B, OFF0 + 1 + base - 5, 6),
                    op=mybir.AluOpType.add)
                nc.scalar.activation(z[:, 6 * T:12 * T], z[:, 6 * T:12 * T],
                                     Silu, bias=sb1, scale=1.0)
                zs[t] = (z, zv)

            def msgAll(t):
                # all 12 message matmuls (+U1a for interior) in one burst
                _, zv = zs.pop(t)
                boundary = t == 0 or t == NCH - 1
                if t == 0:
                    for s, d in enumerate(SEG_ORDER):
                        if d < 0:
                            nc.vector.memset(zv[:, s, 0:-d], 0.0)
                if t == NCH - 1:
                    for s, d in enumerate(SEG_ORDER):
                        if d > 0:
                            nc.vector.memset(zv[:, s, T - d:T], 0.0)
                if boundary:
                    a_ps = psUX.tile([HID, T], f32, tag="ux")
                    for s in range(12):
                        nc.tensor.matmul(a_ps, sW2s, zv[:, s, :],
                                         start=(s == 0), stop=(s == 11))
                    aps[t] = a_ps
                else:
                    u_ps = psUX.tile([HID, T], f32, tag="ux")
                    nc.tensor.matmul(u_ps, sU1a, hts[t], start=True, stop=False)
                    for s in range(12):
                        nc.tensor.matmul(u_ps, sW2u, zv[:, s, :],
                                         start=False, stop=(s == 11))
                    us[t] = u_ps

            def s2em(t):
                # interior: silu of update-MLP hidden
                s2 = work.tile([HID, T], DT, tag="s2")
                nc.scalar.activation(s2, us.pop(t), Silu, bias=sbu, scale=1.0)
                s2s[t] = s2

            def bfix(t):
                # boundary chunks: explicit agg + count fixup + U1b path
                a_ps = aps.pop(t)
                agg = work.tile([HID, T], DT, tag="agg_sb")
                nc.vector.tensor_copy(agg, a_ps)
                if t == 0:
                    nc.vector.tensor_tensor(
                        out=agg[:, 0:K], in0=a_ps[:, 0:K],
                        in1=sfixf, op=mybir.AluOpType.mult)
                else:
                    nc.vector.tensor_tensor(
                        out=agg[:, T - K:T], in0=a_ps[:, T - K:T],
                        in1=sfixl, op=mybir.AluOpType.mult)
                u_ps = psUX.tile([HID, T], f32, tag="ux")
                nc.tensor.matmul(u_ps, sU1a, hts[t], start=True, stop=False)
                nc.tensor.matmul(u_ps, sU1b, agg, start=False, stop=True)
                s2 = work.tile([HID, T], DT, tag="s2")
                nc.scalar.activation(s2, u_ps, Silu, bias=sbu, scale=1.0)
                s2s[t] = s2

            xps = {}
            x2s = {}

            def tailA(t):
                # x_psum = U2.T @ s2 + Ident @ h
                x_ps = psUX.tile([HID, T], f32, tag="ux")
                nc.tensor.matmul(x_ps, sU2, s2s.pop(t), start=True, stop=False)
                nc.tensor.matmul(x_ps, sIdent, hts.pop(t), start=False, stop=True)
                xps[t] = x_ps

            def tailB_dve(t):
                # x = x_psum + b2u  (plain cast with per-partition bias)
                x_ps = xps.pop(t)
                base = t * T
                x_sb = x_full[:, base:base + T]
                nc.vector.tensor_scalar(out=x_sb, in0=x_ps, scalar1=sb2u,
                                        scalar2=None, op0=mybir.AluOpType.add)
                x2 = work.tile([HID, T], DT, tag="x2")
                nc.vector.tensor_tensor(out=x2, in0=x_sb, in1=x_sb,
                                        op=mybir.AluOpType.mult)
                x2s[t] = x2

            def stats_pe(t):
                # stats rows (quadrant bank): E[x] row i, E[x2] row 32+i of the
                # group's 64-row window.  Groups: chunks 0-15 -> window 0,
                # 16-30 -> window 64 (batched right after chunk 30), 31 ->
                # window 0 reused (its own tiny group).
                base = t * T
                x_sb = x_full[:, base:base + T]
                x2 = x2s.pop(t)
                h_, i_ = t // NHALF, t % NHALF
                first = i_ == 0
                last = i_ == NHALF - 1
                st = st_ps[64 * h_:64 * h_ + 64, :]
                r_e2 = 2 * NHALF + i_
                nc.tensor.matmul(st, ssel[:, HOT - i_:HOT - i_ + 4 * NHALF],
                                 x_sb, start=first, stop=False)
                nc.tensor.matmul(st, ssel[:, HOT - r_e2:HOT - r_e2 + 4 * NHALF],
                                 x2, start=False, stop=last)

            smstash = {}

            def stats_mathA(r0, n):
                # first half of the batch math: moments -> sd
                ex_sb = work.tile([NHALF, T], f32, tag="ex")
                nc.vector.tensor_copy(ex_sb[0:n, :], st_ps[r0:r0 + n, :])
                t1 = work.tile([NHALF, T], f32, tag="t1")
                nc.vector.tensor_tensor(out=t1[0:n, :], in0=ex_sb[0:n, :],
                                        in1=ex_sb[0:n, :],
                                        op=mybir.AluOpType.mult)
                var = work.tile([NHALF, T], f32, tag="var")
                nc.vector.tensor_tensor(
                    out=var[0:n, :], in0=st_ps[r0 + 32:r0 + 32 + n, :],
                    in1=t1[0:n, :], op=mybir.AluOpType.subtract)
                nc.scalar.activation(var[0:n, :], var[0:n, :], Sqrt,
                                     bias=seps[0:n], scale=1.0)
                smstash[r0] = (ex_sb, var)

            def stats_mathB(r0, n, ru):
                # second half: reciprocal + mu*r rows
                ex_sb, var = smstash.pop(r0)
                with nc.allow_low_precision(reason="rstd rows feed fp16 matmuls"):
                    nc.vector.reciprocal(out=ru[0:n, 0:T], in_=var[0:n, :])
                nc.vector.tensor_tensor(out=ru[0:n, T:2 * T], in0=ex_sb[0:n, :],
                                        in1=ru[0:n, 0:T],
                                        op=mybir.AluOpType.mult)

            def stats_math(r0, n, ru):
                # batched per-token LN stats for n chunks at bank window r0
                # E[x] rows to SBUF; E[x2] stays in PSUM (rows r0+32.. —
                # 32-aligned; PSUM+SB operand bases may differ, SB+SB may not)
                ex_sb = work.tile([NHALF, T], f32, tag="ex")
                nc.vector.tensor_copy(ex_sb[0:n, :], st_ps[r0:r0 + n, :])
                t1 = work.tile([NHALF, T], f32, tag="t1")
                nc.vector.tensor_tensor(out=t1[0:n, :], in0=ex_sb[0:n, :],
                                        in1=ex_sb[0:n, :],
                                        op=mybir.AluOpType.mult)
                var = work.tile([NHALF, T], f32, tag="var")
                nc.vector.tensor_tensor(
                    out=var[0:n, :], in0=st_ps[r0 + 32:r0 + 32 + n, :],
                    in1=t1[0:n, :], op=mybir.AluOpType.subtract)
                nc.scalar.activation(var[0:n, :], var[0:n, :], Sqrt,
                                     bias=seps[0:n], scale=1.0)
                with nc.allow_low_precision(reason="rstd rows feed fp16 matmuls"):
                    nc.vector.reciprocal(out=ru[0:n, 0:T], in_=var[0:n, :])
                nc.vector.tensor_tensor(out=ru[0:n, T:2 * T], in0=ex_sb[0:n, :],
                                        in1=ru[0:n, 0:T],
                                        op=mybir.AluOpType.mult)

            p2live = {}

            def pass2_pe(t, pool=None):
                # normalize chunk t, matmul part: p1 = g x r, p2 = g x mu*r
                ru = stpool.tile([1, 2 * T], DT, tag="ru")
                src = ru_sb0 if t < NHALF else ru_sb1
                nc.gpsimd.dma_start(out=ru, in_=src[t % NHALF:t % NHALF + 1, :])
                pool, tg = (psPP, "pp") if pool is None else (psUX, "ux")
                p1 = pool.tile([HID, T], f32, tag=tg)
                nc.tensor.matmul(p1, sg, ru[0:1, 0:T], start=True, stop=True)
                p2 = pool.tile([HID, T], f32, tag=tg)
                nc.tensor.matmul(p2, sg, ru[0:1, T:2 * T], start=True, stop=True)
                p2live[t] = (p1, p2)

            def pass2_dve(t):
                # out = x*p1 + lnb - p2
                base = t * T
                p1, p2 = p2live.pop(t)
                o = opool.tile([HID, T], DT, tag="o")
                nc.vector.tensor_tensor(out=o, in0=x_full[:, base:base + T],
                                        in1=p1, op=mybir.AluOpType.mult)
                nc.vector.scalar_tensor_tensor(
                    out=o, in0=o, scalar=slnb, in1=p2,
                    op0=mybir.AluOpType.add, op1=mybir.AluOpType.subtract)
                if t % 2 == 0:
                    nc.scalar.dma_start(out=outT[:, base:base + T], in_=o)
                else:
                    nc.sync.dma_start(out=outT[:, base:base + T], in_=o)

            # ---------------- fused pipeline ----------------
            # D/E run THREE chunks ahead (emitted at iteration end), so at
            # every iteration start each engine's queue head is ready:
            #   PE : U2+Id(c-2) | stats(c-3) | msgAll(c-1) | p1,p2 | D,E(c+2)
            #   DVE: build(c) x2 | x(c-2), x2(c-2) | o1, o2 | casts(c+2)
            #   ACT: silu(c) | s2(c-1)
            p2q = []
            for t in range(3):
                load_chunk(t)
            phase_de_mm(0)
            phase_de_cast(0)
            phase_de_mm(1)
            phase_de_cast(1)
            for c in range(NCH + 3):
                if c + 2 < NCH:
                    phase_de_mm(c + 2)
                if 2 <= c <= NCH + 1:
                    tailA(c - 2)
                if 3 <= c:
                    stats_pe(c - 3)
                if 1 <= c <= NCH:
                    msgAll(c - 1)
                pj = p2q.pop(0) if (p2q and c >= NHALF + 3) else None
                if c < NCH:
                    zbuild(c)
                if c + 2 < NCH:
                    phase_de_cast(c + 2)
                if 2 <= c <= NCH + 1:
                    tailB_dve(c - 2)
                if 1 <= c <= NCH:
                    t = c - 1
                    if t == 0 or t == NCH - 1:
                        bfix(t)
                    else:
                        s2em(t)
                if c == NCH + 1:
                    # all silus emitted; prefetch the sqrt table set
                    nc.scalar.activation(scr, scr, Sqrt, bias=0.0, scale=1.0)
                if c - 3 == NHALF - 1:
                    stats_math(0, NHALF, ru_sb0)
                    p2q.extend(range(NHALF))
                if c + 3 < NCH:
                    load_chunk(c + 3)
                if pj is not None:
                    pass2_pe(pj)
                    pass2_dve(pj)
            stats_math(64, NHALF, ru_sb1)
            rest = list(range(NHALF, NCH)) + p2q
            pass2_pe(rest[0])
            for i, t in enumerate(rest):
                if i + 1 < len(rest):
                    # ux banks are free in the tail: alternate pools for
                    # deeper grid-matmul pipelining
                    pass2_pe(rest[i + 1], psUX if (i % 2 == 0) else None)
                pass2_dve(t)

    nc.compile()
    return nc


def _get_compiled(dt_name):
    global _compiled
    if _compiled is None:
        from concourse import mybir
        dt = {"bf16": mybir.dt.bfloat16, "fp16": mybir.dt.float16,
              "fp32": mybir.dt.float32}[dt_name]
        _compiled = _build_bass(dt)
    return _compiled


DT_NAME = "fp16"


def _sel_band(act_np):
    sel = np.zeros((HID, 2 * 2 * NCH - 1), dtype=np.float32)
    sel[:, 2 * NCH - 1] = 1.0 / HID
    return sel.astype(act_np)


def kernel(**inputs):
    from concourse.bass_utils import run_bass_kernel_spmd

    h = np.asarray(inputs["h"], dtype=np.float32)
    coord = np.asarray(inputs["coord"], dtype=np.float32)
    msg_w1 = np.asarray(inputs["msg_w1"], dtype=np.float32)
    msg_b1 = np.asarray(inputs["msg_b1"], dtype=np.float32)
    msg_w2 = np.asarray(inputs["msg_w2"], dtype=np.float32)
    msg_b2 = np.asarray(inputs["msg_b2"], dtype=np.float32)
    upd_w1 = np.asarray(inputs["upd_w1"], dtype=np.float32)
    upd_b1 = np.asarray(inputs["upd_b1"], dtype=np.float32)
    upd_w2 = np.asarray(inputs["upd_w2"], dtype=np.float32)
    upd_b2 = np.asarray(inputs["upd_b2"], dtype=np.float32)
    ln_g = np.asarray(inputs["ln_g"], dtype=np.float32)
    ln_b = np.asarray(inputs["ln_b"], dtype=np.float32)

    import ml_dtypes
    act_np = {"bf16": ml_dtypes.bfloat16, "fp16": np.float16,
              "fp32": np.float32}[DT_NAME]

    W1a = msg_w1[:HID]
    W1b = msg_w1[HID:2 * HID]
    w1c = msg_w1[2 * HID]
    U1b_f = upd_w1[HID:2 * HID]
    bias_u = upd_b1 + msg_b2 @ U1b_f
    W2s = msg_w2 / (2.0 * K)
    W2u = W2s @ U1b_f

    idx = np.arange(N)
    count = (np.minimum(idx, K) + np.minimum(N - 1 - idx, K)).astype(np.float32)
    fix = (2.0 * K) / count
    fixf = fix[:K].reshape(1, K).astype(np.float32)
    fixl = fix[N - K:].reshape(1, K).astype(np.float32)

    const = {
        "W1a": np.ascontiguousarray(W1a, dtype=act_np),
        "W1b": np.ascontiguousarray(W1b, dtype=act_np),
        "w1c": np.ascontiguousarray(w1c.reshape(1, HID), dtype=act_np),
        "w1cn": np.ascontiguousarray(-w1c.reshape(1, HID), dtype=act_np),
        "W2s": np.ascontiguousarray(W2s, dtype=act_np),
        "W2u": np.ascontiguousarray(W2u, dtype=act_np),
        "U1a": np.ascontiguousarray(upd_w1[:HID], dtype=act_np),
        "U1b": np.ascontiguousarray(U1b_f, dtype=act_np),
        "U2": np.ascontiguousarray(upd_w2, dtype=act_np),
        "b1c": np.ascontiguousarray(msg_b1.reshape(HID, 1), dtype=np.float32),
        "buc": np.ascontiguousarray(bias_u.reshape(HID, 1), dtype=np.float32),
        "b2uc": np.ascontiguousarray(upd_b2.reshape(HID, 1), dtype=np.float32),
        "lnbc": np.ascontiguousarray(ln_b.reshape(HID, 1), dtype=np.float32),
        "g_row": np.ascontiguousarray(ln_g.reshape(1, HID), dtype=act_np),
        "ident": np.ascontiguousarray(np.eye(HID), dtype=act_np),
        "fixf": fixf,
        "fixl": fixl,
        "selb": _sel_band(act_np),
    }

    in_maps = []
    for b in range(B):
        m = dict(const)
        m["hT"] = np.ascontiguousarray(h[b].T, dtype=act_np)
        m["coordR"] = np.ascontiguousarray(coord[b].reshape(1, N), dtype=act_np)
        in_maps.append(m)

    nc = _get_compiled(DT_NAME)
    res = run_bass_kernel_spmd(nc, in_maps, core_ids=list(range(B)))
    global LAST_RESULTS
    LAST_RESULTS = res
    out = np.stack([np.asarray(res.results[b]["outT"], dtype=np.float32).T
                    for b in range(B)])
    return np.ascontiguousarray(out)
